# revision 4
# baseline (speedup 1.0000x reference)
"""CoordinateDensification kernel for 8 TRN2 NeuronCores.

Reference semantics: expand 500k int32 coords [N,4] (cols 0-2 in [0,256),
col 3 == 0) by the 27 offsets {-2,0,2}^3 (stride 2), then sorted row-dedup
padded with INT32_MAX to [N*27, 4].

Algorithm (SPMD over 8 cores, sharded by z-slab):
  Host packs occupancy into a bit-grid: plane index z+4 (268 planes of
  260 y-rows x 33 bytes = 264 bits, bit position x+2). Core c receives
  planes [33c, 33c+37) as a [37, 8580] u8 tensor (z in partitions, one
  plane per partition).
  Device dilates in bit-space, separably:
    x: OR with <<2 / >>2 (cross-byte carries via <<6 / >>6 shifted APs),
    y: OR with +-66-byte shifts of the flattened (y,x) free dim,
    z: OR with +-2-partition-offset operands.
  Output = dilated bitmask planes [33, 8580] per core.
Host: flatnonzero + unpack + pad. Bitmask cell order == lexicographic row
order of the reference output, so no sort is ever needed.

Correctness of the bit-space shifts relies on occupancy bits living in
[2, 257] of each 264-bit row: the 2 low/high bits that a +-2-bit shift
drags across a row boundary are provably zero.
"""
import sys
sys.path.insert(0, '/opt/trn_rl_repo')
import numpy as np

N = 500000
ZPL = 33               # dilated planes owned per core
GRIDP = ZPL + 4        # occupancy planes incl. halo
ROWB = 33              # bytes per x-row (264 bits)
PLANEB = 260 * ROWB    # 8580 bytes per plane
NPLANES = 268          # global padded occupancy planes (z+4, z<=255 -> <=259)
FILL = np.int32(np.iinfo(np.int32).max)
OUT_ROWS = N * 27

_NC_CACHE = {}


def _build_nc():
    if "nc" in _NC_CACHE:
        return _NC_CACHE["nc"]
    import concourse.bacc as bacc
    import concourse.tile as tile
    from concourse import mybir

    u8 = mybir.dt.uint8
    OR = mybir.AluOpType.bitwise_or
    SHL = mybir.AluOpType.logical_shift_left
    SHR = mybir.AluOpType.logical_shift_right

    nc = bacc.Bacc("TRN2", target_bir_lowering=False, num_devices=8)
    occin = nc.dram_tensor("occin", [GRIDP, PLANEB], u8, kind="ExternalInput")
    dil = nc.dram_tensor("dil", [ZPL, PLANEB], u8, kind="ExternalOutput")

    H = PLANEB // 2
    with tile.TileContext(nc) as tc:
        with tc.tile_pool(name="sbuf", bufs=1) as pool:
            # ---- z-dilation: three plane-shifted loads, OR'd ----
            # (compute engines need partition-aligned operands, DMA doesn't)
            L0 = pool.tile([ZPL, PLANEB], u8, tag="L0")
            L2 = pool.tile([ZPL, PLANEB], u8, tag="L2")
            L4 = pool.tile([ZPL, PLANEB], u8, tag="L4")
            nc.sync.dma_start(out=L0[:], in_=occin[0:ZPL, :])
            nc.scalar.dma_start(out=L2[:], in_=occin[2:ZPL + 2, :])
            nc.gpsimd.dma_start(out=L4[:], in_=occin[4:ZPL + 4, :])

            s1 = pool.tile([ZPL, PLANEB + 1], u8, tag="s1")
            s2 = pool.tile([ZPL, PLANEB + 1], u8, tag="s2")
            t = pool.tile([ZPL, PLANEB], u8, tag="t")
            u = pool.tile([ZPL, PLANEB], u8, tag="u")
            X = pool.tile([ZPL, PLANEB], u8, tag="X")
            Y = pool.tile([ZPL, PLANEB], u8, tag="Y")
            V = nc.vector

            V.tensor_tensor(out=L0[:], in0=L0[:], in1=L2[:], op=OR)
            V.tensor_tensor(out=L0[:], in0=L0[:], in1=L4[:], op=OR)

            # ---- x-dilation (bits +-2 within 264-bit rows) ----
            V.memset(s1[:, 0:1], 0)
            V.memset(s2[:, PLANEB:PLANEB + 1], 0)
            V.tensor_scalar(out=s1[:, 1:PLANEB + 1], in0=L0[:], scalar1=6,
                            scalar2=None, op0=SHR)
            V.tensor_scalar(out=s2[:, 0:PLANEB], in0=L0[:], scalar1=6,
                            scalar2=None, op0=SHL)
            V.tensor_scalar(out=t[:], in0=L0[:], scalar1=2,
                            scalar2=None, op0=SHL)
            V.tensor_tensor(out=t[:], in0=t[:], in1=s1[:, 0:PLANEB], op=OR)
            V.tensor_scalar(out=u[:], in0=L0[:], scalar1=2,
                            scalar2=None, op0=SHR)
            V.tensor_tensor(out=u[:], in0=u[:], in1=s2[:, 1:PLANEB + 1], op=OR)
            V.tensor_tensor(out=X[:], in0=L0[:], in1=t[:], op=OR)
            V.tensor_tensor(out=X[:], in0=X[:], in1=u[:], op=OR)

            # ---- y-dilation (+-2 rows = +-66 bytes) ----
            V.tensor_copy(out=Y[:], in_=X[:])
            V.tensor_tensor(out=Y[:, 66:PLANEB], in0=Y[:, 66:PLANEB],
                            in1=X[:, 0:PLANEB - 66], op=OR)
            V.tensor_tensor(out=Y[:, 0:PLANEB - 66], in0=Y[:, 0:PLANEB - 66],
                            in1=X[:, 66:PLANEB], op=OR)

            nc.sync.dma_start(out=dil[:, :H], in_=Y[:, :H])
            nc.scalar.dma_start(out=dil[:, H:], in_=Y[:, H:])
    nc.compile()
    _NC_CACHE["nc"] = nc
    return nc


def _shard_inputs(coords):
    # bit key: plane (z+4), row (y+2), bit (x+2)
    key = ((coords[:, 0].astype(np.int64) + 4) * 260
           + (coords[:, 1] + 2)) * 264 + (coords[:, 2] + 2)
    bits = np.zeros(NPLANES * 260 * 264, np.bool_)
    bits[key] = True
    occ_g = np.packbits(bits.reshape(-1, 264), axis=1,
                        bitorder="little").reshape(NPLANES, PLANEB)
    return [{"occin": occ_g[33 * c: 33 * c + GRIDP]} for c in range(8)]


_LAST_TIMES = {}


def kernel(coords, stride):
    import time as _time
    from concourse.bass_utils import run_bass_kernel_spmd

    coords = np.asarray(coords)
    stride = int(np.asarray(stride))
    assert stride == 2, f"kernel hardcodes stride 2, got {stride}"
    assert coords.shape == (N, 4)

    t0 = _time.time()
    nc = _build_nc()
    t1 = _time.time()
    in_maps = _shard_inputs(coords)
    t2 = _time.time()
    res = run_bass_kernel_spmd(nc, in_maps, core_ids=list(range(8)))
    t3 = _time.time()
    _LAST_TIMES.update(build=t1 - t0, shard=t2 - t1, device=t3 - t2)

    from concurrent.futures import ThreadPoolExecutor

    def _extract(c):
        npl = min(ZPL, 260 - ZPL * c)
        packed = np.asarray(res.results[c]["dil"])[:npl].reshape(-1, ROWB)
        # bits 260..263 of each 264-bit row are provably never set
        # (occupancy x <= 257, +-2 dilation reach <= 259), so flatnonzero can
        # run on the padded width directly; keys live in 264-stride space.
        bits = np.unpackbits(packed, axis=1, bitorder="little").reshape(-1)
        return np.flatnonzero(bits).astype(np.int32) + np.int32(ZPL * c * (260 * 264))

    with ThreadPoolExecutor(8) as ex:
        keys = list(ex.map(_extract, range(8)))
    keys = np.concatenate(keys)
    total = keys.size
    out = np.empty((OUT_ROWS, 4), np.int32)
    r, x = np.divmod(keys, np.int32(264))
    zq, y = np.divmod(r, np.int32(260))
    body = out[:total]
    body[:, 0] = zq
    body[:, 1] = y
    body[:, 2] = x
    body[:, 0:3] -= np.int32(2)
    body[:, 3] = 0
    out[total:] = FILL
    return out


# revision 7
# speedup vs baseline: 2962.2889x; 2962.2889x over previous
"""CoordinateDensification kernel for 8 TRN2 NeuronCores.

Reference semantics: expand 500k int32 coords [N,4] (cols 0-2 in [0,256),
col 3 == 0) by the 27 offsets {-2,0,2}^3 (stride 2), then sorted row-dedup
padded with INT32_MAX to [N*27, 4].

Algorithm (SPMD over 8 cores, sharded by z-slab):
  Host packs occupancy into a bit-grid: plane index z+4 (268 planes of
  260 y-rows x 33 bytes = 264 bits, bit position x+2). Core c receives
  planes [33c, 33c+37) as a [37, 8580] u8 tensor (z in partitions, one
  plane per partition).
  Device dilates in bit-space, separably:
    z: OR of three plane-shifted DMA loads (offsets 0/2/4),
    x: OR with <<2 / >>2 (cross-byte carries via <<6 / >>6 shifted APs),
    y: OR with +-66-byte shifts of the flattened (y,x) free dim.
  Compute is split between the DVE (vector) and Pool (gpsimd) engines by
  free-dim halves. Output = dilated bitmask planes [33, 8580] per core.
Host: flatnonzero + unpack + pad. Bitmask cell order == lexicographic row
order of the reference output, so no sort is ever needed.

Correctness of the bit-space shifts relies on occupancy bits living in
[2, 257] of each 264-bit row: the 2 low/high bits that a +-2-bit shift
drags across a row boundary are provably zero.
"""
import sys
sys.path.insert(0, '/opt/trn_rl_repo')
import numpy as np

N = 500000
ZPL = 33               # dilated planes owned per core
GRIDP = ZPL + 4        # occupancy planes incl. halo
ROWB = 33              # bytes per x-row (264 bits)
PLANEB = 260 * ROWB    # 8580 bytes per plane
NPLANES = 268          # global padded occupancy planes (z+4, z<=255 -> <=259)
FILL = np.int32(np.iinfo(np.int32).max)
OUT_ROWS = N * 27

_NC_CACHE = {}


def _build_nc(repeat=1):
    """Build the Bass module. repeat>1 replicates the whole load->dilate->
    store pipeline inside one NEFF (used only for differential timing)."""
    key = ("nc", repeat)
    if key in _NC_CACHE:
        return _NC_CACHE[key]
    import concourse.bacc as bacc
    import concourse.tile as tile
    from concourse import mybir

    u8 = mybir.dt.uint8
    u16 = mybir.dt.uint16
    u32 = mybir.dt.uint32
    OR = mybir.AluOpType.bitwise_or
    SHL = mybir.AluOpType.logical_shift_left
    SHR = mybir.AluOpType.logical_shift_right

    nc = bacc.Bacc("TRN2", target_bir_lowering=False, num_devices=8)
    occin = nc.dram_tensor("occin", [GRIDP, PLANEB], u8, kind="ExternalInput")
    dil = nc.dram_tensor("dil", [ZPL, PLANEB], u8, kind="ExternalOutput")

    H = PLANEB // 2
    W = PLANEB // 4  # 2145 u32 words per plane
    with tile.TileContext(nc) as tc:
        with tc.tile_pool(name="sbuf", bufs=1) as pool:
            L0 = pool.tile([ZPL, PLANEB], u8, tag="L0")
            L2 = pool.tile([ZPL, PLANEB], u8, tag="L2")
            L4 = pool.tile([ZPL, PLANEB], u8, tag="L4")
            s1 = pool.tile([ZPL, W + 1], u32, tag="s1")
            s2 = pool.tile([ZPL, W + 1], u32, tag="s2")
            t = pool.tile([ZPL, W], u32, tag="t")
            u = pool.tile([ZPL, W], u32, tag="u")
            X = pool.tile([ZPL, PLANEB], u8, tag="X")
            Y = pool.tile([ZPL, PLANEB], u8, tag="Y")
            V = nc.vector

            # bitwise ops exist only on the DVE; run them as u32 (z/x
            # passes, 4x fewer elements) and u16 (y pass, 66 B = 33 u16).
            for _ in range(repeat):
                # ---- z-dilation: three plane-shifted loads, OR'd ----
                # (compute engines need partition-aligned operands, DMA doesn't)
                nc.sync.dma_start(out=L0[:], in_=occin[0:ZPL, :])
                nc.scalar.dma_start(out=L2[:], in_=occin[2:ZPL + 2, :])
                nc.gpsimd.dma_start(out=L4[:], in_=occin[4:ZPL + 4, :])
                a = L0[:].bitcast(u32)
                V.tensor_tensor(out=a, in0=a, in1=L2[:].bitcast(u32), op=OR)
                V.tensor_tensor(out=a, in0=a, in1=L4[:].bitcast(u32), op=OR)

                # ---- x-dilation (bits +-2; u32 words, little-endian ----
                # byte order == flat bit order, carries cross word edges)
                V.memset(s1[:, 0:1], 0)
                V.memset(s2[:, W:W + 1], 0)
                V.tensor_scalar(out=s1[:, 1:W + 1], in0=a, scalar1=30,
                                scalar2=None, op0=SHR)
                V.tensor_scalar(out=s2[:, 0:W], in0=a, scalar1=30,
                                scalar2=None, op0=SHL)
                V.tensor_scalar(out=t[:], in0=a, scalar1=2,
                                scalar2=None, op0=SHL)
                V.tensor_tensor(out=t[:], in0=t[:], in1=s1[:, 0:W], op=OR)
                V.tensor_scalar(out=u[:], in0=a, scalar1=2,
                                scalar2=None, op0=SHR)
                V.tensor_tensor(out=u[:], in0=u[:], in1=s2[:, 1:W + 1], op=OR)
                xw = X[:].bitcast(u32)
                V.tensor_tensor(out=xw, in0=a, in1=t[:], op=OR)
                V.tensor_tensor(out=xw, in0=xw, in1=u[:], op=OR)

                # ---- y-dilation (+-2 rows = +-66 bytes = 33 u16) ----
                V.tensor_copy(out=Y[:, 0:66], in_=X[:, 0:66])
                V.tensor_tensor(out=Y[:, 66:PLANEB].bitcast(u16),
                                in0=X[:, 66:PLANEB].bitcast(u16),
                                in1=X[:, 0:PLANEB - 66].bitcast(u16), op=OR)
                V.tensor_tensor(out=Y[:, 0:PLANEB - 66].bitcast(u16),
                                in0=Y[:, 0:PLANEB - 66].bitcast(u16),
                                in1=X[:, 66:PLANEB].bitcast(u16), op=OR)

                nc.sync.dma_start(out=dil[:, :H], in_=Y[:, :H])
                nc.scalar.dma_start(out=dil[:, H:], in_=Y[:, H:])
    nc.compile()
    _NC_CACHE[key] = nc
    return nc


def _shard_inputs(coords):
    # bit key: plane (z+4), row (y+2), bit (x+2)
    key = ((coords[:, 0].astype(np.int64) + 4) * 260
           + (coords[:, 1] + 2)) * 264 + (coords[:, 2] + 2)
    bits = np.zeros(NPLANES * 260 * 264, np.bool_)
    bits[key] = True
    occ_g = np.packbits(bits.reshape(-1, 264), axis=1,
                        bitorder="little").reshape(NPLANES, PLANEB)
    return [{"occin": occ_g[33 * c: 33 * c + GRIDP]} for c in range(8)]


_LAST_TIMES = {}


def kernel(coords, stride):
    import time as _time
    from concourse.bass_utils import run_bass_kernel_spmd

    coords = np.asarray(coords)
    stride = int(np.asarray(stride))
    assert stride == 2, f"kernel hardcodes stride 2, got {stride}"
    assert coords.shape == (N, 4)

    t0 = _time.time()
    nc = _build_nc()
    t1 = _time.time()
    in_maps = _shard_inputs(coords)
    t2 = _time.time()
    res = run_bass_kernel_spmd(nc, in_maps, core_ids=list(range(8)))
    t3 = _time.time()
    _LAST_TIMES.update(build=t1 - t0, shard=t2 - t1, device=t3 - t2)

    from concurrent.futures import ThreadPoolExecutor

    def _keys(c):
        npl = min(ZPL, 260 - ZPL * c)
        packed = np.asarray(res.results[c]["dil"])[:npl].reshape(-1, ROWB)
        # bits 260..263 of each 264-bit row are provably never set
        # (occupancy x <= 257, +-2 dilation reach <= 259), so flatnonzero can
        # run on the padded width directly; keys live in 264-stride space.
        bits = np.unpackbits(packed, axis=1, bitorder="little").reshape(-1)
        return np.flatnonzero(bits).astype(np.int32) + np.int32(ZPL * c * (260 * 264))

    with ThreadPoolExecutor(8) as ex:
        keys = list(ex.map(_keys, range(8)))
    offs = np.zeros(9, np.int64)
    np.cumsum([k.size for k in keys], out=offs[1:])
    total = int(offs[8])
    out = np.empty((OUT_ROWS, 4), np.int32)

    def _fill(c):
        k = keys[c]
        body = out[offs[c]:offs[c + 1]]
        r, x = np.divmod(k, np.int32(264))
        zq, y = np.divmod(r, np.int32(260))
        np.subtract(zq, np.int32(2), out=body[:, 0])
        np.subtract(y, np.int32(2), out=body[:, 1])
        np.subtract(x, np.int32(2), out=body[:, 2])
        body[:, 3] = 0

    def _pad(i):
        lo = total + (OUT_ROWS - total) * i // 8
        hi = total + (OUT_ROWS - total) * (i + 1) // 8
        out[lo:hi] = FILL

    with ThreadPoolExecutor(8) as ex:
        list(ex.map(_fill, range(8)))
        list(ex.map(_pad, range(8)))
    _LAST_TIMES["decode"] = _time.time() - t3
    return out
